# revision 24
# baseline (speedup 1.0000x reference)
"""MoE-LoRA linear layer (top-2 routing) as a Bass/Tile kernel for 8 TRN2 cores.

Sharding: data-parallel over tokens. N = B*S = 8192 tokens -> 1024 per core.
Weights (base_w^T, lora_A packed, lora_B) are replicated across cores.

Routing (logits -> softmax -> top-2 -> renormalized dense gate [N, E]) is
computed on host with the exact same jax CPU ops as the reference: the top-2
selection is discontinuous, and this seed has near-tie tokens, so the
selection must match the reference bit-for-bit. It is 0.3% of the FLOPs.

GEMM operands are fp16 (PSUM accumulation is fp32): |x| < ~6 and |w| < ~0.2,
so fp16's 2^-11 rounding gives ~6e-4 worst-case relative error while doubling
the PE streaming rate vs fp32 and halving the weight DMA.

Per-core device compute (tokens on PSUM partitions, 8 m-tiles of 128 tokens):
  - PE-transpose x tiles (fp16) to get xT [d_in, tok] stationary operands
  - LoRA-A GEMM -> per-expert mids; gate-scale the mids (DVE)
  - LoRA-B GEMM opens each output PSUM accumulation group (it does not
    depend on the streamed base weight), then the base GEMM x @ base_w^T
    accumulates on top; fp32 bias is added on the way out.

The first PREFETCH m-tiles' transpose/LoRA-A work is emitted before the base
GEMMs so the PE has work while base_w^T streams in. x rides the scalar-engine
HWDGE ring, gate/lora_A/bias ride the gpsimd SWDGE ring, so neither queues
behind the 8MB base-weight stream on the sync ring.
"""

import numpy as np

B, S, D, O, E, R = 4, 2048, 2048, 2048, 8, 16
SCALING = 32.0 / 16.0
NCORES = 8
N = B * S
NT = N // NCORES      # tokens per core
MT = NT // 128        # m-tiles per core
KT = D // 128         # k-tiles (contraction over d_in)
NBLK = O // 512       # 512-wide output blocks
ER = E * R            # 128
PREFETCH = 3          # m-tiles of transpose/LoRA-A work emitted up front

_cache = {}


def _build():
    import concourse.bacc as bacc
    import concourse.tile as tile
    import concourse.mybir as mybir

    f32 = mybir.dt.float32
    f16 = mybir.dt.float16

    nc = bacc.Bacc("TRN2", target_bir_lowering=False, debug=False,
                   num_devices=NCORES)
    x_d = nc.dram_tensor("x", [NT, D], f16, kind="ExternalInput")
    wt_d = nc.dram_tensor("wt", [D, O], f16, kind="ExternalInput")
    ra_d = nc.dram_tensor("ra", [128, KT * ER], f16, kind="ExternalInput")
    bc_d = nc.dram_tensor("bc", [ER, O], f16, kind="ExternalInput")
    bias_d = nc.dram_tensor("bias", [1, O], f32, kind="ExternalInput")
    gate_d = nc.dram_tensor("gate", [128, MT * E], f32, kind="ExternalInput")
    id_d = nc.dram_tensor("ident", [128, 128], f16, kind="ExternalInput")
    out_d = nc.dram_tensor("out", [NT, O], f32, kind="ExternalOutput")

    with tile.TileContext(nc) as tc:
        with (
            tc.tile_pool(name="weights", bufs=1) as wpool,
            tc.tile_pool(name="xin", bufs=4) as xpool,
            tc.tile_pool(name="xt", bufs=PREFETCH + 1) as xtpool,
            tc.tile_pool(name="small", bufs=2) as gpool,
            tc.tile_pool(name="outp", bufs=4) as opool,
            tc.tile_pool(name="pt", bufs=2, space="PSUM") as ptpool,
            tc.tile_pool(name="pmid", bufs=2, space="PSUM") as pmidpool,
            tc.tile_pool(name="pout", bufs=1, space="PSUM") as poutpool,
        ):
            x_chunks = {}

            id_sb = wpool.tile([128, 128], f16, tag="ident")
            nc.scalar.dma_start(out=id_sb, in_=id_d[:, :])

            def emit_x(m):
                """DMA one m-tile's x rows (m0 on sync, rest on scalar)."""
                rows = slice(128 * m, 128 * (m + 1))
                x_sb = xpool.tile([128, D], f16, tag="x", name="x_sb")
                eng = nc.sync if m == 0 else nc.scalar
                eng.dma_start(out=x_sb, in_=x_d[rows, :])
                x_chunks[m] = x_sb

            for m in range(PREFETCH):
                emit_x(m)

            # ---- small weights on the gpsimd SWDGE ring ----
            ra_sb = wpool.tile([128, KT, ER], f16, tag="ra")
            nc.gpsimd.dma_start(out=ra_sb,
                                in_=ra_d.rearrange("p (k e) -> p k e", k=KT))
            gate_sb = wpool.tile([128, MT, E], f32, tag="gate")
            nc.gpsimd.dma_start(
                out=gate_sb,
                in_=gate_d.rearrange("p (m e) -> p m e", m=MT))
            bias_sb = wpool.tile([128, O], f32, tag="bias")
            nc.gpsimd.dma_start(out=bias_sb,
                                in_=bias_d[0:1, :].partition_broadcast(128))

            # ---- lora_B on the scalar ring behind the prefetch x tiles ----
            bc_sb = wpool.tile([128, O], f16, tag="bc")
            nc.scalar.dma_start(out=bc_sb, in_=bc_d[:, :])
            # ---- sync ring: x m0 then the 8MB base weight ----
            wt_sb = []
            for k in range(KT):
                t = wpool.tile([128, O], f16, tag=f"wt{k}")
                nc.sync.dma_start(out=t, in_=wt_d[128 * k:128 * (k + 1), :])
                wt_sb.append(t)

            xt_tiles = {}
            gmidT_tiles = {}

            def emit_pre(m):
                """Transposes, LoRA-A GEMM, gate-scale, gmid^T."""
                if m not in x_chunks:
                    emit_x(m)
                xt_sb = xtpool.tile([128, KT, 128], f16, tag="xt",
                                    name="xt_sb")
                x_sb = x_chunks.pop(m)
                for k in range(KT):
                    pt = ptpool.tile([128, 128], f16, tag="pt", name="pt")
                    nc.tensor.transpose(
                        pt, x_sb[:, 128 * k:128 * (k + 1)], id_sb)
                    nc.vector.tensor_copy(out=xt_sb[:, k, :], in_=pt)

                pmid = pmidpool.tile([128, ER], f32, tag="pmid", name="pmid")
                for k in range(KT):
                    nc.tensor.matmul(
                        pmid, xt_sb[:, k, :], ra_sb[:, k, :],
                        start=(k == 0), stop=(k == KT - 1))

                gmid = gpool.tile([128, ER], f16, tag="gmid", name="gmid")
                nc.vector.tensor_mul(
                    gmid.rearrange("p (e r) -> p e r", e=E),
                    pmid.rearrange("p (e r) -> p e r", e=E),
                    gate_sb[:, m, :].unsqueeze(2).broadcast_to([128, E, R]))
                pgt = ptpool.tile([128, 128], f16, tag="pt", name="pgt")
                nc.tensor.transpose(pgt, gmid, id_sb)
                gmidT = gpool.tile([128, 128], f16, tag="gmidT",
                                   name="gmidT", bufs=PREFETCH + 2)
                nc.vector.tensor_copy(out=gmidT, in_=pgt)
                xt_tiles[m] = xt_sb
                gmidT_tiles[m] = gmidT

            def emit_main(m):
                """LoRA-B opens the psum group, base GEMM accumulates."""
                rows = slice(128 * m, 128 * (m + 1))
                xt_sb = xt_tiles.pop(m)
                gmidT = gmidT_tiles.pop(m)
                pouts = [poutpool.tile([128, 512], f32, tag=f"pout{n}",
                                       name=f"pout{n}")
                         for n in range(NBLK)]
                for n in range(NBLK):
                    nc.tensor.matmul(
                        pouts[n], gmidT, bc_sb[:, 512 * n:512 * (n + 1)],
                        start=True, stop=False)
                for k in range(KT):
                    xk = xt_sb[:, k, :]
                    for n in range(NBLK):
                        nc.tensor.matmul(
                            pouts[n], xk,
                            wt_sb[k][:, 512 * n:512 * (n + 1)],
                            start=False, stop=(k == KT - 1))
                for n in range(NBLK):
                    cols = slice(512 * n, 512 * (n + 1))
                    o_sb = opool.tile([128, 512], f32, tag="o", name="o_sb")
                    nc.vector.tensor_add(o_sb, pouts[n], bias_sb[:, cols])
                    nc.sync.dma_start(out=out_d[rows, cols], in_=o_sb)

            for m in range(PREFETCH):
                emit_pre(m)
            for m in range(MT):
                if m + PREFETCH < MT:
                    emit_pre(m + PREFETCH)
                emit_main(m)

    nc.compile()
    return nc


def _get_nc():
    if "nc" not in _cache:
        _cache["nc"] = _build()
    return _cache["nc"]


def _host_gate(x, router_w, router_b):
    """Dense [N, E] top-2 gate, bit-identical to the reference's routing."""
    import jax
    import jax.numpy as jnp

    cpu = jax.devices("cpu")[0]
    with jax.default_device(cpu):
        xj = jnp.asarray(np.asarray(x, dtype=np.float32))
        logits = jnp.einsum("bsd,ed->bse",
                            xj,
                            jnp.asarray(np.asarray(router_w,
                                                   dtype=np.float32)))
        logits = logits + jnp.asarray(np.asarray(router_b, dtype=np.float32))
        probs = jax.nn.softmax(logits.astype(jnp.float32), axis=-1)
        top_vals, top_idx = jax.lax.top_k(probs, 2)
        top_vals = top_vals / jnp.sum(top_vals, axis=-1, keepdims=True)
        flat_idx = np.asarray(top_idx).reshape(N, 2)
        flat_val = np.asarray(top_vals.astype(jnp.float32)).reshape(N, 2)
    gate = np.zeros((N, E), dtype=np.float32)
    np.put_along_axis(gate, flat_idx, flat_val, axis=1)
    return gate


def _prep_in_maps(x, base_w, base_b, router_w, router_b, lora_A, lora_B):
    gate = _host_gate(x, router_w, router_b)

    x = np.asarray(x, dtype=np.float32).reshape(N, D)
    base_w = np.asarray(base_w, dtype=np.float32)
    base_b = np.asarray(base_b, dtype=np.float32)
    lora_A = np.asarray(lora_A, dtype=np.float32)
    lora_B = np.asarray(lora_B, dtype=np.float32)

    x16 = x.astype(np.float16)
    wt = np.ascontiguousarray(base_w.T).astype(np.float16)     # [D, O]
    # lora_A packed partition-major: ra[p, k*ER + e] = lora_A_cat[k*128+p, e]
    a_cat = lora_A.transpose(1, 0, 2).reshape(D, ER)           # [D, ER]
    ra = np.ascontiguousarray(
        a_cat.reshape(KT, 128, ER).transpose(1, 0, 2).reshape(128, KT * ER)
    ).astype(np.float16)
    bc = (lora_B.reshape(ER, O) * np.float32(SCALING)).astype(np.float16)
    bias = base_b.reshape(1, O)
    ident = np.eye(128, dtype=np.float16)

    shared = {"wt": wt, "ra": ra, "bc": bc, "bias": bias, "ident": ident}
    maps = []
    for i in range(NCORES):
        g = gate[NT * i:NT * (i + 1)]                          # [NT, E]
        g = np.ascontiguousarray(
            g.reshape(MT, 128, E).transpose(1, 0, 2).reshape(128, MT * E))
        maps.append(dict(shared, x=x16[NT * i:NT * (i + 1)], gate=g))
    return maps


def _run(in_maps, **kwargs):
    from concourse.bass_utils import run_bass_kernel_spmd
    nc = _get_nc()
    return run_bass_kernel_spmd(nc, in_maps, list(range(NCORES)), **kwargs)


def kernel(x, base_w, base_b, router_w, router_b, lora_A, lora_B):
    in_maps = _prep_in_maps(x, base_w, base_b, router_w, router_b,
                            lora_A, lora_B)
    res = _run(in_maps)
    out = np.concatenate([res.results[i]["out"] for i in range(NCORES)],
                         axis=0)
    return out.reshape(B, S, O)
